# revision 8
# baseline (speedup 1.0000x reference)
"""ConvLRUBlock Trainium2 kernel: 8-core SPMD, H-sharded.

Reference pipeline:
  7x7 spatial conv (circular W pad, edge H pad) -> 1x1 depth conv
  -> RMSNorm(C) -> GLU (w_in) -> diagonal complex LRU scan over L
  -> w_out + residual.

Key transforms done on host:
  * depth conv composes into the spatial conv (both linear):
      w_comb[o,i,ky,kx] = sum_c w_depth[o,c] * w_spatial[c,i,ky,kx]
  * only Re(h) is used downstream, so the LRU scan has a closed form
      h_re[t] = sum_{s<=t} Re(lam^(t-s)) * u[s]
    i.e. a per-channel 32x32 lower-triangular matmul along L.
  * rms_weight and the LRU input normalization gamma fold into w_in.

Device layout (per core, H shard of 16 rows, 2 chunks of 8 rows):
  * conv uses row-pair packing: SBUF x tile [128p = (row-parity, c), 7 rows, 134 cols]
    with odd-global rows in partitions 0-63 and even in 64-127; 28 stationary
    [128,128] matrices (4 row-offsets x 7 kx taps) accumulate the full conv for
    4 output-row-pairs per matmul (N=512) into PSUM.
"""

import sys

sys.path.insert(0, "/opt/trn_rl_repo")

import numpy as np
import ml_dtypes

import concourse.bacc as bacc
import concourse.tile as tile
import concourse.mybir as mybir
from concourse.bass_utils import run_bass_kernel_spmd

F = mybir.dt.float32
BF = mybir.dt.bfloat16

B, C, L, H, W = 1, 64, 32, 128, 128
K = 7
PAD = 3
NCORES = 8
HC = H // NCORES          # 16 output rows per core
R = 8                     # chunk rows
NCHUNK = HC // R          # 2
WP = W + 2 * PAD          # 134
NPOS = R * W              # positions per (chunk, l) = 1024
NG = 16                   # scan channel groups; channel c = j*16 + g

_CACHE = {}


def _build_host_weights(w_spatial, w_depth, b_depth, rms_weight, w_in, b_in,
                        w_out, b_out, nu_log, theta_log):
    f32 = np.float32
    w_comb = np.einsum("oc,cikl->oikl", w_depth.astype(np.float64),
                       w_spatial.astype(np.float64)).astype(f32)

    # conv stationaries: 4 row-offsets (d = 2*di-3) x 7 kx
    wconv = np.zeros((4 * K, 128, 128), f32)
    for di in range(4):
        d = 2 * di - 3
        for kx in range(K):
            S = wconv[di * K + kx]
            for jk in range(2):          # input row parity block
                for jm in range(2):      # output row parity block
                    ky = d + 3 + jk - jm
                    if 0 <= ky < K:
                        # S[jk*64+ci, jm*64+co] = w_comb[co, ci, ky, kx]
                        S[jk * 64:jk * 64 + 64, jm * 64:jm * 64 + 64] = \
                            w_comb[:, :, ky, kx].T
    lam_mod = np.exp(-np.exp(nu_log.astype(np.float64)))
    lam_arg = np.exp(theta_log.astype(np.float64))
    lam = lam_mod * np.exp(1j * lam_arg)
    gamma = np.sqrt(1.0 - lam_mod ** 2)

    # scan stationaries: group g holds channels c = j*16 + g, j in 0..4
    # lhsT[j*32+s, j*32+t] = Re(lam_c^(t-s)) for t >= s
    powers = np.stack([np.real(lam ** k) for k in range(L)], 0)  # [L, C]
    wscan = np.zeros((NG, 128, 128), f32)
    for g in range(NG):
        for j in range(4):
            c = j * 16 + g
            blk = np.zeros((L, L), f32)
            for t in range(L):
                blk[0:t + 1, t] = powers[t::-1, c]  # blk[s, t] = Re(lam^(t-s))
            wscan[g, j * 32:(j + 1) * 32, j * 32:(j + 1) * 32] = blk

    win_eff = (w_in * rms_weight[None, :]).astype(f32)          # [128, 64]
    win_eff[0:64] *= gamma[:, None].astype(f32)
    win_t = win_eff.T.copy()                                    # [64, 128] lhsT
    wout_t = w_out.T.copy().astype(f32)                         # [64, 64] lhsT

    wones = np.zeros((128, 2), f32)
    wones[0:64, 0] = 1.0
    wones[64:128, 1] = 1.0

    bin1 = (b_in[0:64] * gamma).astype(f32).reshape(64, 1)
    bin2 = np.tile(b_in[64:128].astype(f32), 2).reshape(128, 1)
    bdep = np.tile(b_depth.astype(f32), 2).reshape(128, 1)
    bout = b_out.astype(f32).reshape(64, 1)

    bf = ml_dtypes.bfloat16
    return dict(
        wconv=wconv.astype(bf), wscan=wscan.astype(bf), win=win_t.astype(bf),
        wout=wout_t.astype(bf), wones=wones.astype(bf),
        bin1=bin1, bin2=bin2, bdep=bdep, bout=bout,
    )


def _build_program():
    nc = bacc.Bacc("TRN2", target_bir_lowering=False, debug=False)
    xs = nc.declare_dram_parameter("xs", [C, L, HC + 2 * PAD, W], F, isOutput=False)
    wconv = nc.declare_dram_parameter("wconv", [4 * K, 128, 128], BF, isOutput=False)
    wscan = nc.declare_dram_parameter("wscan", [NG, 128, 128], BF, isOutput=False)
    win = nc.declare_dram_parameter("win", [64, 128], BF, isOutput=False)
    wout = nc.declare_dram_parameter("wout", [64, 64], BF, isOutput=False)
    wones = nc.declare_dram_parameter("wones", [128, 2], BF, isOutput=False)
    bin1 = nc.declare_dram_parameter("bin1", [64, 1], F, isOutput=False)
    bin2 = nc.declare_dram_parameter("bin2", [128, 1], F, isOutput=False)
    bdep = nc.declare_dram_parameter("bdep", [128, 1], F, isOutput=False)
    bout = nc.declare_dram_parameter("bout", [64, 1], F, isOutput=False)
    y = nc.declare_dram_parameter("y", [C, L, HC, W], F, isOutput=True)

    with tile.TileContext(nc) as tc:
        with (
            tc.tile_pool(name="const", bufs=1) as const,
            tc.tile_pool(name="xf", bufs=2) as xf_pool,
            tc.tile_pool(name="xb", bufs=2) as xb_pool,
            tc.tile_pool(name="mid", bufs=2) as mid,
            tc.tile_pool(name="ubuf", bufs=1) as ubuf,
            tc.tile_pool(name="hbuf", bufs=1) as hbuf,
            tc.tile_pool(name="outp", bufs=2) as outp,
            tc.tile_pool(name="ps_conv", bufs=2, space="PSUM") as ps_conv,
            tc.tile_pool(name="ps_ms", bufs=2, space="PSUM") as ps_ms,
            tc.tile_pool(name="ps_z", bufs=2, space="PSUM") as ps_z,
            tc.tile_pool(name="ps_big", bufs=1, space="PSUM") as ps_big,
            tc.tile_pool(name="dscr", bufs=2, space="DRAM") as dscr,
        ):
            wconv_sb = const.tile([128, 4 * K, 128], BF)
            nc.sync.dma_start(out=wconv_sb[:], in_=wconv.rearrange("t p w -> p t w"))
            wscan_sb = const.tile([128, NG, 128], BF)
            nc.sync.dma_start(out=wscan_sb[:], in_=wscan.rearrange("t p w -> p t w"))
            win_sb = const.tile([128, 128], BF)
            nc.sync.dma_start(out=win_sb[0:64, :], in_=win[:])
            nc.sync.dma_start(out=win_sb[64:128, :], in_=win[:])
            wout_sb = const.tile([64, 64], BF)
            nc.sync.dma_start(out=wout_sb[:], in_=wout[:])
            wones_sb = const.tile([128, 2], BF)
            nc.sync.dma_start(out=wones_sb[:], in_=wones[:])
            bin1_sb = const.tile([64, 1], F)
            nc.sync.dma_start(out=bin1_sb[:], in_=bin1[:])
            bin2_sb = const.tile([128, 1], F)
            nc.sync.dma_start(out=bin2_sb[:], in_=bin2[:])
            bdep_sb = const.tile([128, 1], F)
            nc.sync.dma_start(out=bdep_sb[:], in_=bdep[:])
            bout_sb = const.tile([64, 1], F)
            nc.sync.dma_start(out=bout_sb[:], in_=bout[:])
            eps_sb = const.tile([2, 1], F)
            nc.vector.memset(eps_sb[:], 1e-6)

            for ch in range(NCHUNK):
                r0 = ch * R
                u_ch = ubuf.tile([128, NG, NPOS], BF)
                for l in range(L):
                    # ---- load x slice: odd global rows -> partitions 0:64 ----
                    xt = xf_pool.tile([128, K, WP], F)
                    for half, rb in ((0, r0), (64, r0 + 1)):
                        rows = xs[:, l, rb:rb + 2 * K - 1:2, :]
                        nc.sync.dma_start(
                            out=xt[half:half + 64, :, PAD:PAD + W], in_=rows)
                        nc.sync.dma_start(
                            out=xt[half:half + 64, :, 0:PAD],
                            in_=xs[:, l, rb:rb + 2 * K - 1:2, W - PAD:W])
                        nc.sync.dma_start(
                            out=xt[half:half + 64, :, PAD + W:WP],
                            in_=xs[:, l, rb:rb + 2 * K - 1:2, 0:PAD])
                    xb = xb_pool.tile([128, K, WP], BF)
                    nc.vector.tensor_copy(xb[:], xt[:])

                    # ---- conv: 28 accumulating matmuls ----
                    pconv = ps_conv.tile([128, 4, W], F)
                    n_mm = 4 * K
                    for di in range(4):
                        for kx in range(K):
                            i_mm = di * K + kx
                            nc.tensor.matmul(
                                pconv[:],
                                wconv_sb[:, i_mm, :],
                                xb[:, di:di + 4, kx:kx + W],
                                start=(i_mm == 0), stop=(i_mm == n_mm - 1))

                    # ---- evac + b_depth bias (f32 -> bf16) ----
                    ce = mid.tile([128, 4 * W], BF)
                    nc.scalar.activation(
                        out=ce[:], in_=pconv[:].rearrange("p a b -> p (a b)"),
                        func=mybir.ActivationFunctionType.Identity,
                        bias=bdep_sb[:], scale=1.0)

                    # ---- RMS stats: mean over channels via ones-matmul ----
                    sq = mid.tile([128, 4 * W], BF)
                    nc.gpsimd.tensor_mul(sq[:], ce[:], ce[:])
                    pms = ps_ms.tile([2, 4 * W], F)
                    nc.tensor.matmul(pms[:], wones_sb[:], sq[:], start=True, stop=True)
                    srt = mid.tile([2, 4 * W], F)
                    nc.scalar.activation(
                        out=srt[:], in_=pms[:],
                        func=mybir.ActivationFunctionType.Sqrt,
                        bias=eps_sb[:], scale=1.0 / 64)
                    inv = mid.tile([2, 4 * W], BF)
                    with nc.allow_low_precision(reason="1/rms stored bf16; 0.4% ok at 2e-2 gate"):
                        nc.vector.reciprocal(inv[:], srt[:])
                    inv_dr = dscr.tile([2, 4 * W], BF)
                    nc.sync.dma_start(out=inv_dr[:], in_=inv[:])
                    rb16 = mid.tile([128, 4 * W], BF)
                    nc.sync.dma_start(out=rb16[0:64, :],
                                      in_=inv_dr[0:1, :].broadcast_to([64, 4 * W]))
                    nc.sync.dma_start(out=rb16[64:128, :],
                                      in_=inv_dr[1:2, :].broadcast_to([64, 4 * W]))
                    cn = mid.tile([128, 4 * W], BF)
                    nc.vector.tensor_mul(cn[:], ce[:], rb16[:])

                    # ---- w_in + GLU, per row-parity ----
                    for par in range(2):
                        pz = ps_z.tile([128, 4 * W], F, tag="pz")
                        nc.tensor.matmul(
                            pz[:], win_sb[64 * par:64 * par + 64, :],
                            cn[64 * par:64 * par + 64, :],
                            start=True, stop=True)
                        sig = mid.tile([64, 4 * W], F, tag=f"sig{par}")
                        nc.scalar.activation(
                            out=sig[:], in_=pz[64:128, :],
                            func=mybir.ActivationFunctionType.Sigmoid,
                            bias=bin2_sb[64:128, :], scale=1.0)
                        us = mid.tile([64, 4 * W], BF, tag=f"us{par}")
                        nc.vector.scalar_tensor_tensor(
                            out=us[:], in0=pz[0:64, :], scalar=bin1_sb[:],
                            in1=sig[:], op0=mybir.AluOpType.add,
                            op1=mybir.AluOpType.mult)
                        # scatter into u_ch: partition j*32+l, cols (g, par*512+s)
                        for j in range(4):
                            nc.sync.dma_start(
                                out=u_ch[j * 32 + l:j * 32 + l + 1, :,
                                         512 * par:512 * par + 512],
                                in_=us[16 * j:16 * j + 16, :])

                # ---- LRU scan: 16 block-diag triangular matmuls ----
                h_ch = hbuf.tile([64, L, NPOS], BF)
                for g in range(NG):
                    pb = ps_big.tile([128, NPOS], F, tag="pbig")
                    for hf in range(2):
                        nc.tensor.matmul(
                            pb[:, 512 * hf:512 * hf + 512],
                            wscan_sb[:, g, :],
                            u_ch[:, g, 512 * hf:512 * hf + 512],
                            start=True, stop=True)
                    hs = outp.tile([128, NPOS], BF, tag="hs")
                    nc.scalar.copy(hs[:], pb[:])
                    for j in range(4):
                        nc.sync.dma_start(
                            out=h_ch[j * 16 + g:j * 16 + g + 1, :, :],
                            in_=hs[32 * j:32 * j + 32, :])

                # ---- w_out + residual ----
                for l in range(L):
                    pb = ps_big.tile([128, NPOS], F, tag="pbig")
                    for hf in range(2):
                        nc.tensor.matmul(
                            pb[0:64, 512 * hf:512 * hf + 512], wout_sb[:],
                            h_ch[:, l, 512 * hf:512 * hf + 512],
                            start=True, stop=True)
                    xr = outp.tile([64, 2, 4, W], F, tag="xr")
                    for par in range(2):
                        nc.sync.dma_start(
                            out=xr[:, par, :, :],
                            in_=xs[:, l, r0 + PAD + par:r0 + PAD + R - 1 + par:2, :])
                    yo = outp.tile([64, 2, 4, W], F, tag="yo")
                    nc.vector.scalar_tensor_tensor(
                        out=yo[:].rearrange("c a b w -> c (a b w)"),
                        in0=pb[0:64, :], scalar=bout_sb[:],
                        in1=xr[:].rearrange("c a b w -> c (a b w)"),
                        op0=mybir.AluOpType.add, op1=mybir.AluOpType.add)
                    for par in range(2):
                        nc.sync.dma_start(
                            out=y[:, l, r0 + par:r0 + R - 1 + par:2, :],
                            in_=yo[:, par, :, :])
    nc.compile()
    return nc


def kernel(x, w_spatial, w_depth, b_depth, rms_weight, w_in, b_in, w_out,
           b_out, nu_log, theta_log):
    x = np.asarray(x, np.float32)
    wts = _build_host_weights(
        np.asarray(w_spatial, np.float32), np.asarray(w_depth, np.float32),
        np.asarray(b_depth, np.float32), np.asarray(rms_weight, np.float32),
        np.asarray(w_in, np.float32), np.asarray(b_in, np.float32),
        np.asarray(w_out, np.float32), np.asarray(b_out, np.float32),
        np.asarray(nu_log, np.float32), np.asarray(theta_log, np.float32))

    if "nc" not in _CACHE:
        _CACHE["nc"] = _build_program()
    nc = _CACHE["nc"]

    xpad = np.pad(x[0], ((0, 0), (0, 0), (PAD, PAD), (0, 0)), mode="edge")
    in_maps = []
    for core in range(NCORES):
        m = dict(wts)
        m["xs"] = np.ascontiguousarray(xpad[:, :, HC * core:HC * core + HC + 2 * PAD, :])
        in_maps.append(m)

    res = run_bass_kernel_spmd(nc, in_maps, list(range(NCORES)))
    out = np.empty((B, C, L, H, W), np.float32)
    for core in range(NCORES):
        out[0, :, :, HC * core:HC * core + HC, :] = res.results[core]["y"]
    return out
